# revision 1
# baseline (speedup 1.0000x reference)
"""KStepRGCN Trainium2 kernel: 8-core SPMD Bass/Tile implementation.

Sharding: nodes partitioned into 8 dst-slices (graph-partition style).
Each core aggregates messages for its dst-slice via dma_gather (bf16 rows
from a replicated node-feature table) + PE one-hot segment-sum matmuls,
then applies the per-relation basis-decomposed transforms in fp32.
Between layers the updated slices are AllGathered into the next table.
"""

import sys

sys.path.insert(0, "/opt/trn_rl_repo")

import numpy as np
import ml_dtypes

BF16 = ml_dtypes.bfloat16

# problem constants (hardcoded per harness contract)
N, E, D, R, B, K = 50000, 600000, 128, 3, 3, 3
NCORES = 8
LO_LIMIT = 32768
SEGC = 8  # chunks per gather segment; 8*128 idxs = 64 descs/SDMA lane (single_packet ceiling)


class Cfg:
    def __init__(self, n=N, e=E, ncores=NCORES):
        assert n % ncores == 0
        self.n, self.e, self.ncores = n, e, ncores
        self.ns = n // ncores                 # real nodes per slice
        self.tpc = (self.ns + 127) // 128     # col tiles per relation
        self.nsp = self.tpc * 128             # padded slice
        self.trows = ncores * self.nsp        # table rows
        self.nblk = R * self.tpc              # psum blocks per layer


def _preprocess(cfg, edge_index, edge_attr):
    """Build per-core static schedule + host tensors.

    Returns (sched, per_core list of dict, perms list).
    """
    src = np.asarray(edge_index[0], dtype=np.int64)
    dst = np.asarray(edge_index[1], dtype=np.int64)
    attr = np.asarray(edge_attr, dtype=np.int64)
    ns, nsp, tpc, nc_ = cfg.ns, cfg.nsp, cfg.tpc, cfg.ncores

    deg_total = np.bincount(dst, minlength=cfg.n)  # mean divisor (all relations)

    # --- per-core node permutation: snake-balance total degree across blocks
    perms = []
    for c in range(nc_):
        deg_local = deg_total[c * ns:(c + 1) * ns]
        order = np.argsort(-deg_local, kind="stable")
        i = np.arange(ns)
        g, o = i // tpc, i % tpc
        b = np.where(g % 2 == 0, o, tpc - 1 - o)      # snake over bins
        perm = np.empty(ns, dtype=np.int64)
        perm[order] = b * 128 + g
        perms.append(perm)

    # table row for each global node
    row_of = np.empty(cfg.n, dtype=np.int64)
    for c in range(nc_):
        row_of[c * ns:(c + 1) * ns] = c * nsp + perms[c]

    lo_lim = min(LO_LIMIT, cfg.trows)
    hi_rows = cfg.trows - lo_lim

    # --- per-core edge bucketing by (stream, block), fully vectorized
    core_of = dst // ns
    streams_pc = []   # per core: (lo=(row, bl, colw, rank), hi=(...))
    max_lo = 1
    max_hi = 0
    for c in range(nc_):
        m = core_of == c
        s_c, v_c, r_c = src[m], dst[m] - c * ns, attr[m]
        pos = perms[c][v_c]
        bl = r_c * tpc + pos // 128
        colw = pos % 128
        row = row_of[s_c]
        is_lo = row < lo_lim
        parts = []
        for sel, base in ((is_lo, 0), (~is_lo, lo_lim)):
            blv, rv, cv = bl[sel], row[sel] - base, colw[sel]
            order = np.argsort(blv, kind="stable")
            blv, rv, cv = blv[order], rv[order], cv[order]
            cnt = np.bincount(blv, minlength=cfg.nblk)
            start = np.concatenate(([0], np.cumsum(cnt)))
            rank = np.arange(len(blv)) - start[blv]
            parts.append((rv, blv, cv, rank, cnt))
        streams_pc.append(parts)
        max_lo = max(max_lo, int(np.ceil(parts[0][4].max() / 128)) if parts[0][4].size else 1)
        max_hi = max(max_hi, int(np.ceil(parts[1][4].max() / 128)) if parts[1][4].size and parts[1][4].max() else 0)

    cpb_lo, cpb_hi = max(1, max_lo), max_hi
    nlo_chunks = cfg.nblk * cpb_lo
    nhi_chunks = cfg.nblk * cpb_hi
    nlo_seg = (nlo_chunks + SEGC - 1) // SEGC
    nhi_seg = (nhi_chunks + SEGC - 1) // SEGC if nhi_chunks else 0

    def build_stream(part, cpb, nseg):
        """idx [128, nseg*SEGC*8] i16 ; S [128, nseg*SEGC*128] bf16"""
        rv, blv, cv, rank, _ = part
        tot = nseg * SEGC
        idx_flat = np.zeros(tot * 128, dtype=np.int16)
        S = np.zeros((128, tot * 128), dtype=np.float32)
        if len(rv):
            ch = blv * cpb + rank // 128
            epos = rank % 128
            idx_flat[ch * 128 + epos] = rv.astype(np.int16)
            S[epos, ch * 128 + cv] = 1.0
        idx_w = np.tile(
            idx_flat.reshape(nseg, SEGC * 8, 16).transpose(0, 2, 1)
            .reshape(nseg, 16, SEGC * 8).transpose(1, 0, 2).reshape(16, tot * 8),
            (8, 1)).astype(np.int16)
        return idx_w, S.astype(BF16)

    per_core = []
    for c in range(nc_):
        lo_p, hi_p = streams_pc[c]
        il, sl = build_stream(lo_p, cpb_lo, nlo_seg)
        if nhi_seg:
            ih, sh = build_stream(hi_p, cpb_hi, nhi_seg)
        else:
            ih = np.zeros((128, SEGC * 8), dtype=np.int16)
            sh = np.zeros((128, SEGC * 128), dtype=BF16)
        per_core.append(dict(idx_lo=il, S_lo=sl, idx_hi=ih, S_hi=sh))

    sched = dict(cpb_lo=cpb_lo, cpb_hi=cpb_hi, nlo_seg=nlo_seg, nhi_seg=nhi_seg,
                 lo_lim=lo_lim, hi_rows=hi_rows)
    inv_cnt = 1.0 / np.maximum(deg_total, 1).astype(np.float32)
    return sched, per_core, perms, inv_cnt


def _build_program(cfg, sched, k_layers=K, prelu_a=0.25):
    """Emit the Bass program. Returns (nc, names of IO tensors)."""
    from concourse import bacc, mybir
    import concourse.tile as tile

    f32, bf16, i16 = mybir.dt.float32, mybir.dt.bfloat16, mybir.dt.int16
    Alu = mybir.AluOpType
    tpc, nsp, nblk, trows = cfg.tpc, cfg.nsp, cfg.nblk, cfg.trows
    cpb_lo, cpb_hi = sched["cpb_lo"], sched["cpb_hi"]
    nlo_seg, nhi_seg = sched["nlo_seg"], sched["nhi_seg"]
    lo_lim, hi_rows = sched["lo_lim"], sched["hi_rows"]

    nc = bacc.Bacc("TRN2", target_bir_lowering=False, debug=False,
                   num_devices=cfg.ncores)

    # --- IO tensors
    x_table = nc.dram_tensor("x_table", [trows, D], bf16, kind="ExternalInput")
    x_own = nc.dram_tensor("x_own", [128, nsp], f32, kind="ExternalInput")
    w_sw = nc.dram_tensor("w_sw", [128, k_layers * R * D], f32, kind="ExternalInput")
    root_sw = nc.dram_tensor("root_sw", [128, k_layers * D], f32, kind="ExternalInput")
    bias_in = nc.dram_tensor("bias_in", [1, k_layers * D], f32, kind="ExternalInput")
    ident_in = nc.dram_tensor("ident_in", [128, 128], f32, kind="ExternalInput")
    invc_in = nc.dram_tensor("invc_in", [128, tpc], f32, kind="ExternalInput")
    idx_lo_in = nc.dram_tensor("idx_lo", [128, max(1, nlo_seg) * SEGC * 8], i16,
                               kind="ExternalInput")
    s_lo_in = nc.dram_tensor("s_lo", [128, max(1, nlo_seg) * SEGC * 128], bf16,
                             kind="ExternalInput")
    idx_hi_in = nc.dram_tensor("idx_hi", [128, max(1, nhi_seg) * SEGC * 8], i16,
                               kind="ExternalInput")
    s_hi_in = nc.dram_tensor("s_hi", [128, max(1, nhi_seg) * SEGC * 128], bf16,
                             kind="ExternalInput")
    out_own = nc.dram_tensor("out_own", [nsp, D], f32, kind="ExternalOutput")


    # internal tables for AllGather
    ag_in = nc.dram_tensor("ag_in", [nsp, D], bf16, kind="Internal")
    tables = [x_table]
    for i in range(k_layers - 1):
        tables.append(nc.dram_tensor(f"table{i + 1}", [trows, D], bf16,
                                     kind="Internal", addr_space="Shared"))

    rg = [list(range(cfg.ncores))]

    from contextlib import ExitStack

    with tile.TileContext(nc) as tc, ExitStack() as ctx:
        const = ctx.enter_context(tc.tile_pool(name="const", bufs=1))
        w_t = const.tile([128, k_layers * R * D], f32, tag="w")
        root_t = const.tile([128, k_layers * D], f32, tag="root")
        bias_t = const.tile([1, k_layers * D], f32, tag="bias")
        ones_t = const.tile([1, 128], f32, tag="ones")
        ident_t = const.tile([128, 128], f32, tag="ident")
        invc_t = const.tile([128, tpc], f32, tag="invc")
        h_own = const.tile([128, nsp], f32, tag="h_own")
        a_T = const.tile([128, nblk * 128], f32, tag="a_T")
        idx_lo_t = const.tile([128, max(1, nlo_seg) * SEGC * 8], i16, tag="ixl")
        idx_hi_t = const.tile([128, max(1, nhi_seg) * SEGC * 8], i16, tag="ixh")
        hbf = const.tile([128, nsp], bf16, tag="hbf")

        nc.sync.dma_start(w_t[:], w_sw.ap())
        nc.sync.dma_start(root_t[:], root_sw.ap())
        nc.sync.dma_start(bias_t[:], bias_in.ap())
        nc.sync.dma_start(ident_t[:], ident_in.ap())
        nc.sync.dma_start(invc_t[:], invc_in.ap())
        nc.sync.dma_start(h_own[:], x_own.ap())
        nc.sync.dma_start(idx_lo_t[:], idx_lo_in.ap())
        if nhi_seg:
            nc.sync.dma_start(idx_hi_t[:], idx_hi_in.ap())
        nc.vector.memset(ones_t[:], 1.0)

        msg_lo = ctx.enter_context(tc.tile_pool(name="msg_lo", bufs=2))
        msg_hi = ctx.enter_context(tc.tile_pool(name="msg_hi", bufs=2))
        sp_lo = ctx.enter_context(tc.tile_pool(name="sp_lo", bufs=2))
        sp_hi = ctx.enter_context(tc.tile_pool(name="sp_hi", bufs=2))
        pblk = ctx.enter_context(tc.tile_pool(name="pblk", bufs=2, space="PSUM"))
        pout = ctx.enter_context(tc.tile_pool(name="pout", bufs=2, space="PSUM"))
        proot = ctx.enter_context(tc.tile_pool(name="proot", bufs=2, space="PSUM"))
        ptr_p = ctx.enter_context(tc.tile_pool(name="ptr", bufs=2, space="PSUM"))
        hT_pool = ctx.enter_context(tc.tile_pool(name="hT", bufs=2))
        tmp_pool = ctx.enter_context(tc.tile_pool(name="tmp", bufs=2))

        # staged SWDGE path: auto-trigger dma_gather crashes this runtime,
        # prepare_only + trigger_dma works. Persistent sems, cumulative targets.
        prep_sem = ctx.enter_context(nc.semaphore())
        dma_sem = ctx.enter_context(nc.semaphore())
        gcount = [0]

        def emit_gather(mt, in_ap, idxs_ap):
            gcount[0] += 1
            g = gcount[0]
            with tc.tile_critical():
                nc.gpsimd.dma_gather(
                    out_ap=mt[:], in_ap=in_ap, idxs_ap=idxs_ap,
                    num_idxs=SEGC * 128, num_idxs_reg=SEGC * 128, elem_size=D,
                    prepare_only=True, sem=dma_sem).then_inc(prep_sem, 1)
                nc.gpsimd.wait_ge(prep_sem, g)
                nc.gpsimd.trigger_dma(count=1)
                nc.gpsimd.wait_ge(dma_sem, 16 * g)

        # gather segments must be emitted in consumption order (blocks read
        # lo and hi tiles interleaved; pools have finite bufs)
        seg_order = sorted(
            [("lo", s, s * SEGC // cpb_lo) for s in range(nlo_seg)]
            + ([("hi", s, s * SEGC // cpb_hi) for s in range(nhi_seg)]
               if nhi_seg else []),
            key=lambda t: (t[2], t[0] == "hi"))

        for k in range(k_layers):
            table = tables[k]
            # ---- gather segments + S loads, in consumption order
            lo_tiles, lo_S = {}, {}
            hi_tiles, hi_S = {}, {}
            for stream, s, _ in seg_order:
                if stream == "lo":
                    mt = msg_lo.tile([128, SEGC, D], bf16, tag="m")
                    emit_gather(mt, table.ap()[0:lo_lim, :],
                                idx_lo_t[:, s * SEGC * 8:(s + 1) * SEGC * 8])
                    st = sp_lo.tile([128, SEGC * 128], bf16, tag="s")
                    nc.sync.dma_start(st[:], s_lo_in.ap()[:, s * SEGC * 128:(s + 1) * SEGC * 128])
                    lo_tiles[s], lo_S[s] = mt, st
                else:
                    mt = msg_hi.tile([128, SEGC, D], bf16, tag="m")
                    emit_gather(mt, table.ap()[lo_lim:trows, :],
                                idx_hi_t[:, s * SEGC * 8:(s + 1) * SEGC * 8])
                    st = sp_hi.tile([128, SEGC * 128], bf16, tag="s")
                    nc.sync.dma_start(st[:], s_hi_in.ap()[:, s * SEGC * 128:(s + 1) * SEGC * 128])
                    hi_tiles[s], hi_S[s] = mt, st

            # ---- segment-sum into A^T blocks
            n_mm = cpb_lo + (cpb_hi if nhi_seg else 0)
            for bl in range(nblk):
                pb = pblk.tile([128, 128], f32, tag="pb")
                mm = 0
                for j in range(cpb_lo):
                    L = bl * cpb_lo + j
                    s, pos = divmod(L, SEGC)
                    nc.tensor.matmul(pb[:], lhsT=lo_tiles[s][:, pos, :],
                                     rhs=lo_S[s][:, pos * 128:(pos + 1) * 128],
                                     start=(mm == 0), stop=(mm == n_mm - 1))
                    mm += 1
                if nhi_seg:
                    for j in range(cpb_hi):
                        L = bl * cpb_hi + j
                        s, pos = divmod(L, SEGC)
                        nc.tensor.matmul(pb[:], lhsT=hi_tiles[s][:, pos, :],
                                         rhs=hi_S[s][:, pos * 128:(pos + 1) * 128],
                                         start=(mm == 0), stop=(mm == n_mm - 1))
                        mm += 1
                nc.vector.tensor_copy(a_T[:, bl * 128:(bl + 1) * 128], pb[:])

            # ---- transform per col-tile
            for t in range(tpc):
                po = pout.tile([128, 128], f32, tag="po")
                pr = proot.tile([128, 128], f32, tag="pr")
                pt = ptr_p.tile([128, 128], f32, tag="pt")
                for r in range(R):
                    bl = r * tpc + t
                    nc.tensor.matmul(po[:], lhsT=a_T[:, bl * 128:(bl + 1) * 128],
                                     rhs=w_t[:, (k * R + r) * D:(k * R + r + 1) * D],
                                     start=(r == 0), stop=(r == R - 1))
                nc.tensor.transpose(pt[:], h_own[:, t * 128:(t + 1) * 128], ident_t[:])
                hT = hT_pool.tile([128, 128], f32, tag="h")
                nc.vector.tensor_copy(hT[:], pt[:])
                nc.tensor.matmul(pr[:], lhsT=hT[:], rhs=root_t[:, k * D:(k + 1) * D],
                                 start=True, stop=False)
                nc.tensor.matmul(pr[:], lhsT=ones_t[:], rhs=bias_t[:, k * D:(k + 1) * D],
                                 start=False, stop=True)
                tt = tmp_pool.tile([128, 128], f32, tag="t")
                # hw: only one PSUM operand per DVE op -> two steps
                nc.vector.tensor_scalar(tt[:], po[:], invc_t[:, t:t + 1], None,
                                        Alu.mult)
                dst_sl = h_own[:, t * 128:(t + 1) * 128]
                if k < k_layers - 1:
                    pre = tmp_pool.tile([128, 128], f32, tag="t2")
                    nc.vector.tensor_add(pre[:], tt[:], pr[:])
                    nc.vector.scalar_tensor_tensor(dst_sl, pre[:], prelu_a, pre[:],
                                                   Alu.mult, Alu.max)
                else:
                    nc.vector.tensor_add(dst_sl, tt[:], pr[:])

            # ---- export: cast + AllGather (not after last layer)
            if k < k_layers - 1:
                nc.vector.tensor_copy(hbf[:], h_own[:])
                nc.sync.dma_start(
                    ag_in.ap().rearrange("(t p) f -> p t f", p=128),
                    hbf[:].rearrange("p (t f) -> p t f", f=D))
                nc.gpsimd.collective_compute(
                    "AllGather", Alu.bypass, replica_groups=rg,
                    ins=[ag_in.ap()], outs=[tables[k + 1].ap()])

        nc.sync.dma_start(out_own.ap().rearrange("(t p) f -> p t f", p=128),
                          h_own[:].rearrange("p (t f) -> p t f", f=D))

    nc.compile()
    return nc


def _host_tensors(cfg, sched, per_core, perms, inv_cnt, x, basis, att, root, bias,
                  k_layers=K):
    """Build in_maps for all cores."""
    ns, nsp, tpc = cfg.ns, cfg.nsp, cfg.tpc
    # relation weights W[k] = einsum('rb,bio->rio', att[k], basis[k])
    W = np.einsum("krb,kbio->krio", att.astype(np.float32),
                  basis.astype(np.float32))[:k_layers]  # [k,R,D,D]
    root = root[:k_layers]
    bias = bias[:k_layers]
    w_sw = np.ascontiguousarray(
        W.transpose(2, 0, 1, 3).reshape(D, k_layers * R * D)).astype(np.float32)
    root_sw = np.ascontiguousarray(
        root.transpose(1, 0, 2).reshape(D, k_layers * D)).astype(np.float32)
    bias_in = bias.reshape(1, k_layers * D).astype(np.float32)
    ident = np.eye(128, dtype=np.float32)

    # global bf16 table [trows, D]
    table = np.zeros((cfg.trows, D), dtype=BF16)
    for c in range(cfg.ncores):
        sl = x[c * ns:(c + 1) * ns].astype(BF16)
        rowpos = c * nsp + perms[c]
        table[rowpos] = sl

    in_maps = []
    for c in range(cfg.ncores):
        x_own = np.zeros((128, nsp), dtype=np.float32)
        invc = np.ones((128, tpc), dtype=np.float32)
        inv_perm = np.empty(nsp, dtype=np.int64)
        inv_perm.fill(-1)
        for v in range(ns):
            inv_perm[perms[c][v]] = v
        for t in range(tpc):
            for p in range(128):
                v = inv_perm[t * 128 + p]
                if v >= 0:
                    x_own[p, t * 128:(t + 1) * 128] = x[c * ns + v]
                    invc[p, t] = inv_cnt[c * ns + v]
        pc = per_core[c]
        in_maps.append(dict(
            x_table=table, x_own=x_own, w_sw=w_sw, root_sw=root_sw,
            bias_in=bias_in, ident_in=ident, invc_in=invc,
            idx_lo=pc["idx_lo"], s_lo=np.ascontiguousarray(pc["S_lo"]),
            idx_hi=pc["idx_hi"], s_hi=np.ascontiguousarray(pc["S_hi"])))
    return in_maps


def _run(cfg, x, edge_index, edge_attr, basis, att, root, bias, prelu_a,
         k_layers=K, trace=False):
    from concourse.bass_utils import run_bass_kernel_spmd

    sched, per_core, perms, inv_cnt = _preprocess(cfg, edge_index, edge_attr)
    nc = _build_program(cfg, sched, k_layers,
                        float(np.asarray(prelu_a).ravel()[0]))
    in_maps = _host_tensors(cfg, sched, per_core, perms, inv_cnt,
                            np.asarray(x, dtype=np.float32),
                            np.asarray(basis), np.asarray(att),
                            np.asarray(root), np.asarray(bias), k_layers)
    res = run_bass_kernel_spmd(nc, in_maps, core_ids=list(range(cfg.ncores)),
                               trace=trace)
    out = np.empty((cfg.n, D), dtype=np.float32)
    for c in range(cfg.ncores):
        rows = res.results[c]["out_own"]  # [nsp, D] permuted
        out[c * cfg.ns:(c + 1) * cfg.ns] = rows[perms[c]]
    return out, res


def kernel(x, edge_index, edge_attr, basis, att, root, bias, prelu_a):
    cfg = Cfg()
    out, _ = _run(cfg, x, edge_index, edge_attr, basis, att, root, bias, prelu_a)
    return out



# revision 9
# speedup vs baseline: 24.4868x; 24.4868x over previous
"""KStepRGCN Trainium2 kernel: 8-core SPMD Bass/Tile implementation.

Sharding: nodes partitioned into 8 dst-slices (graph-partition style).
Each core aggregates messages for its dst-slice via pipelined dma_gather
(bf16 rows from a replicated node-feature table) + PE one-hot segment-sum
matmuls. The one-hot S matrices are generated on-chip (DVE iota-compare
against per-edge column indices) with the mean divisor folded into the
one-hot values, so the root/bias terms accumulate into the same PSUM
group. Between layers the updated slices are AllGathered into the next
table.
"""

import sys

sys.path.insert(0, "/opt/trn_rl_repo")

import numpy as np
import ml_dtypes

BF16 = ml_dtypes.bfloat16

# problem constants (hardcoded per harness contract)
N, E, D, R, B, K = 50000, 600000, 128, 3, 3, 3
NCORES = 8
LO_LIMIT = 32768
SEGC = 8        # chunks per gather segment (64 descs/SDMA lane ceiling)
MSG_BUFS = 6    # in-flight gather segments per stream (pipeline depth)
S_BUFS = 6
LOOKAHEAD = 4   # gather segments emitted ahead of the consumer wait


class Cfg:
    def __init__(self, n=N, e=E, ncores=NCORES):
        assert n % ncores == 0
        self.n, self.e, self.ncores = n, e, ncores
        self.ns = n // ncores                 # real nodes per slice
        self.tpc = (self.ns + 127) // 128     # col tiles per relation
        self.nsp = self.tpc * 128             # padded slice
        self.trows = ncores * self.nsp        # table rows
        self.nblk = R * self.tpc              # psum blocks per layer


def _wrap_idx(idx_flat, nseg):
    """[nseg*SEGC*128] -> wrapped [128, nseg*SEGC*8] int16."""
    tot = nseg * SEGC
    return np.tile(
        idx_flat.reshape(nseg, SEGC * 8, 16).transpose(0, 2, 1)
        .reshape(nseg, 16, SEGC * 8).transpose(1, 0, 2).reshape(16, tot * 8),
        (8, 1)).astype(np.int16)


def _preprocess(cfg, edge_index, edge_attr):
    """Build the uniform (cross-core) static schedule + per-core host data.

    Schedule: per (block, stream) chunk counts = max over cores, chunks
    packed densely per stream in block order into SEGC-chunk gather
    segments.
    """
    src = np.asarray(edge_index[0], dtype=np.int64)
    dst = np.asarray(edge_index[1], dtype=np.int64)
    attr = np.asarray(edge_attr, dtype=np.int64)
    ns, nsp, tpc, nc_, nblk = cfg.ns, cfg.nsp, cfg.tpc, cfg.ncores, cfg.nblk

    deg_total = np.bincount(dst, minlength=cfg.n)
    inv_cnt = 1.0 / np.maximum(deg_total, 1).astype(np.float32)

    # --- per-core node permutation: snake-balance total degree across bins
    perms = []
    for c in range(nc_):
        deg_local = deg_total[c * ns:(c + 1) * ns]
        order = np.argsort(-deg_local, kind="stable")
        i = np.arange(ns)
        g, o = i // tpc, i % tpc
        b = np.where(g % 2 == 0, o, tpc - 1 - o)      # snake over bins
        perm = np.empty(ns, dtype=np.int64)
        perm[order] = b * 128 + g
        perms.append(perm)

    row_of = np.empty(cfg.n, dtype=np.int64)
    for c in range(nc_):
        row_of[c * ns:(c + 1) * ns] = c * nsp + perms[c]

    lo_lim = min(LO_LIMIT, cfg.trows)
    hi_rows = cfg.trows - lo_lim
    nstreams = 2 if hi_rows > 0 else 1

    # --- per-core edge bucketing by (block, stream)
    core_of = dst // ns
    edges_pc = []   # per core per stream: (row_rel, bl, colw, invc_e) sorted by bl
    cnt = np.zeros((nc_, nblk, 2), dtype=np.int64)
    for c in range(nc_):
        m = core_of == c
        s_c, v_c, r_c = src[m], dst[m] - c * ns, attr[m]
        pos = perms[c][v_c]
        bl = r_c * tpc + pos // 128
        colw = pos % 128
        row = row_of[s_c]
        ive = inv_cnt[dst[m]]
        is_lo = row < lo_lim
        parts = []
        for sidx, (sel, base) in enumerate(((is_lo, 0), (~is_lo, lo_lim))):
            blv, rv, cw, iv = bl[sel], row[sel] - base, colw[sel], ive[sel]
            order = np.argsort(blv, kind="stable")
            blv, rv, cw, iv = blv[order], rv[order], cw[order], iv[order]
            np.add.at(cnt[c, :, sidx], blv, 1)
            parts.append((rv, blv, cw, iv))
        edges_pc.append(parts)

    # --- uniform chunk counts per (block, stream): max over cores
    nch = np.ceil(cnt / 128.0).astype(np.int64).max(axis=0)  # [nblk, 2]
    # guard: every block needs >= 1 chunk so its psum group is written
    empty = nch.sum(axis=1) == 0
    nch[empty, 0] = 1
    if nstreams == 1:
        nch[:, 1] = 0

    qoff = np.zeros((nblk, 2), dtype=np.int64)  # chunk offset within stream
    qoff[:, 0] = np.cumsum(nch[:, 0]) - nch[:, 0]
    qoff[:, 1] = np.cumsum(nch[:, 1]) - nch[:, 1]
    nch_s = [int(nch[:, 0].sum()), int(nch[:, 1].sum())]
    nseg = [(nch_s[0] + SEGC - 1) // SEGC,
            (nch_s[1] + SEGC - 1) // SEGC if nch_s[1] else 0]

    # segment emission order: by (first-use block, stream)
    seg_first_use = []
    for s in range(2):
        for g in range(nseg[s]):
            q0 = g * SEGC
            # first block whose chunk range covers q0 (or follows it)
            fub = int(np.searchsorted(qoff[:, s] + nch[:, s], q0 + 1))
            seg_first_use.append((fub, s, g))
    seg_order = [(s, g) for _, s, g in sorted(seg_first_use)]

    # --- per-core tensors: wrapped idx + cv + invce per stream
    per_core = []
    for c in range(nc_):
        dat = {}
        for s in range(nstreams):
            if nseg[s] == 0:
                continue
            tot = nseg[s] * SEGC
            idx_flat = np.zeros(tot * 128, dtype=np.int16)
            cv = np.full((128, tot), 255.0, dtype=np.float32)
            ive_a = np.ones((128, tot), dtype=np.float32)
            rv, blv, cw, iv = edges_pc[c][s]
            if len(rv):
                start = np.zeros(nblk, dtype=np.int64)
                cnt_c = np.bincount(blv, minlength=nblk)
                start[1:] = np.cumsum(cnt_c)[:-1]
                rank = np.arange(len(blv)) - start[blv]
                q = qoff[blv, s] + rank // 128
                p = rank % 128
                idx_flat[q * 128 + p] = rv.astype(np.int16)
                cv[p, q] = cw
                ive_a[p, q] = iv
            dat[f"idx{s}"] = _wrap_idx(idx_flat, nseg[s])
            dat[f"cv{s}"] = cv.astype(np.float32)
            dat[f"ivc{s}"] = ive_a.astype(np.float32)
        per_core.append(dat)

    sched = dict(nch=nch, qoff=qoff, nseg=nseg, lo_lim=lo_lim,
                 hi_rows=hi_rows, nstreams=nstreams, seg_order=seg_order)
    return sched, per_core, perms, inv_cnt


def _build_program(cfg, sched, k_layers=K, prelu_a=0.25, n_iter=1):
    from concourse import bacc, mybir
    import concourse.tile as tile

    f32, bf16, i16 = mybir.dt.float32, mybir.dt.bfloat16, mybir.dt.int16
    Alu = mybir.AluOpType
    Act = mybir.ActivationFunctionType
    tpc, nsp, nblk, trows = cfg.tpc, cfg.nsp, cfg.nblk, cfg.trows
    nch, qoff = sched["nch"], sched["qoff"]
    nseg, nstreams = sched["nseg"], sched["nstreams"]
    lo_lim, hi_rows = sched["lo_lim"], sched["hi_rows"]
    seg_order = sched["seg_order"]

    nc = bacc.Bacc("TRN2", target_bir_lowering=False, debug=False,
                   num_devices=cfg.ncores)

    # --- IO tensors
    x_table = nc.dram_tensor("x_table", [trows, D], bf16, kind="ExternalInput")
    x_own = nc.dram_tensor("x_own", [128, nsp], f32, kind="ExternalInput")
    w_sw = nc.dram_tensor("w_sw", [128, k_layers * R * D], bf16,
                          kind="ExternalInput")
    root_sw = nc.dram_tensor("root_sw", [128, k_layers * D], bf16,
                             kind="ExternalInput")
    bias_in = nc.dram_tensor("bias_in", [1, k_layers * D], bf16,
                             kind="ExternalInput")
    ident_in = nc.dram_tensor("ident_in", [128, 128], f32, kind="ExternalInput")
    iota_in = nc.dram_tensor("iota_in", [128, 128], bf16, kind="ExternalInput")
    idx_in, cv_in, ivc_in = [None, None], [None, None], [None, None]
    for s in range(nstreams):
        if nseg[s]:
            idx_in[s] = nc.dram_tensor(f"idx{s}", [128, nseg[s] * SEGC * 8],
                                       i16, kind="ExternalInput")
            cv_in[s] = nc.dram_tensor(f"cv{s}", [128, nseg[s] * SEGC], f32,
                                      kind="ExternalInput")
            ivc_in[s] = nc.dram_tensor(f"ivc{s}", [128, nseg[s] * SEGC], f32,
                                       kind="ExternalInput")
    out_own = nc.dram_tensor("out_own", [nsp, D], f32, kind="ExternalOutput")

    # internal tables for AllGather
    ag_in = nc.dram_tensor("ag_in", [nsp, D], bf16, kind="Internal")
    tables = [x_table]
    for i in range(k_layers - 1):
        tables.append(nc.dram_tensor(f"table{i + 1}", [trows, D], bf16,
                                     kind="Internal", addr_space="Shared"))

    rg = [list(range(cfg.ncores))]

    from contextlib import ExitStack

    with tile.TileContext(nc) as tc, ExitStack() as ctx:
        const = ctx.enter_context(tc.tile_pool(name="const", bufs=1))
        w_t = const.tile([128, k_layers * R * D], bf16, tag="w")
        root_t = const.tile([128, k_layers * D], bf16, tag="root")
        bias_t = const.tile([1, k_layers * D], bf16, tag="bias")
        ones_t = const.tile([1, 128], bf16, tag="ones")
        ident_t = const.tile([128, 128], f32, tag="ident")
        iota_t = const.tile([128, 128], bf16, tag="iota")
        h_own = const.tile([128, nsp], f32, tag="h_own")
        a_T = const.tile([128, nblk * 128], bf16, tag="a_T")
        hbf = const.tile([128, nsp], bf16, tag="hbf")
        idx_t, cv_t, ivc_t = [None, None], [None, None], [None, None]
        for s in range(nstreams):
            if nseg[s]:
                idx_t[s] = const.tile([128, nseg[s] * SEGC * 8], i16,
                                      name=f"idxt{s}", tag=f"ix{s}")
                cv_t[s] = const.tile([128, nseg[s] * SEGC], f32,
                                     name=f"cvt{s}", tag=f"cv{s}")
                ivc_t[s] = const.tile([128, nseg[s] * SEGC], f32,
                                      name=f"ivct{s}", tag=f"iv{s}")
                nc.sync.dma_start(idx_t[s][:], idx_in[s].ap())
                nc.sync.dma_start(cv_t[s][:], cv_in[s].ap())
                nc.sync.dma_start(ivc_t[s][:], ivc_in[s].ap())

        nc.sync.dma_start(w_t[:], w_sw.ap())
        nc.sync.dma_start(root_t[:], root_sw.ap())
        nc.sync.dma_start(bias_t[:], bias_in.ap())
        nc.sync.dma_start(ident_t[:], ident_in.ap())
        nc.sync.dma_start(iota_t[:], iota_in.ap())
        nc.vector.memset(ones_t[:], 1.0)

        msg_pools = [
            ctx.enter_context(tc.tile_pool(name=f"msg{s}", bufs=MSG_BUFS))
            for s in range(nstreams)]
        s_pools = [
            ctx.enter_context(tc.tile_pool(name=f"sp{s}", bufs=S_BUFS))
            for s in range(nstreams)]
        pblk = ctx.enter_context(tc.tile_pool(name="pblk", bufs=4,
                                              space="PSUM"))
        pout = ctx.enter_context(tc.tile_pool(name="pout", bufs=2,
                                              space="PSUM"))
        ptr_p = ctx.enter_context(tc.tile_pool(name="ptr", bufs=2,
                                               space="PSUM"))
        hT_pool = ctx.enter_context(tc.tile_pool(name="hT", bufs=2))

        # pipelined SWDGE gathers: rotating per-slot completion semaphores;
        # consumers (PE) wait on the slot sem, prep/trigger never wait for
        # data. (auto-trigger dma_gather crashes this runtime; staged
        # prepare_only + trigger works.)
        prep_sem = ctx.enter_context(nc.semaphore("prep_sem"))
        slot_sems = [[ctx.enter_context(nc.semaphore(f"dsem{s}_{i}"))
                      for i in range(MSG_BUFS)] for s in range(nstreams)]
        prep_count = [0]
        # slot index tracks the msg pool's round-robin buffer assignment
        # (one tile() call per emission), so a slot sem never has two
        # outstanding gathers: prep of emission e waits (pool WAR dep) for
        # the consumers of emission e-MSG_BUFS, which waited on this sem.
        emis_count = [0, 0]
        seg_slot = [{}, {}]   # (s, seg) -> (slot, use_idx) for current layer

        def emit_gather(s, seg, mt, table):
            if s == 0:
                in_ap = table.ap()[0:lo_lim, :]
            else:
                in_ap = table.ap()[lo_lim:trows, :]
            slot = emis_count[s] % MSG_BUFS
            uses = emis_count[s] // MSG_BUFS + 1
            emis_count[s] += 1
            sem = slot_sems[s][slot]
            prep_count[0] += 1
            seg_slot[s][seg] = (slot, uses)
            with tc.tile_critical():
                nc.gpsimd.dma_gather(
                    out_ap=mt[:], in_ap=in_ap,
                    idxs_ap=idx_t[s][:, seg * SEGC * 8:(seg + 1) * SEGC * 8],
                    num_idxs=SEGC * 128, num_idxs_reg=SEGC * 128, elem_size=D,
                    prepare_only=True, sem=sem).then_inc(prep_sem, 1)
                nc.gpsimd.wait_ge(prep_sem, prep_count[0])
                nc.gpsimd.trigger_dma(count=1)

        for it in range(n_iter):
            nc.sync.dma_start(h_own[:], x_own.ap())
            for k in range(k_layers):
                table = tables[k]
                tiles = {}
                waited = set()
                seg_slot[0].clear()
                seg_slot[1].clear()

                def emit_segment(s, seg):
                    mt = msg_pools[s].tile([128, SEGC, D], bf16, tag="m")
                    emit_gather(s, seg, mt, table)
                    st = s_pools[s].tile([128, SEGC * 128], bf16, tag="s")
                    for j in range(SEGC):
                        q = seg * SEGC + j
                        nc.vector.tensor_scalar(
                            st[:, j * 128:(j + 1) * 128], iota_t[:],
                            cv_t[s][:, q:q + 1], ivc_t[s][:, q:q + 1],
                            Alu.is_equal, Alu.mult)
                    tiles[(s, seg)] = (mt, st)

                emit_ptr = [0]

                def emit_ahead(upto_idx):
                    while emit_ptr[0] <= upto_idx and emit_ptr[0] < len(seg_order):
                        s, g = seg_order[emit_ptr[0]]
                        emit_segment(s, g)
                        emit_ptr[0] += 1

                seg_idx = {sg: i for i, sg in enumerate(seg_order)}

                # ---- segment-sum into a_T blocks
                for bl in range(nblk):
                    chunks = [(0, int(qoff[bl, 0]) + j)
                              for j in range(int(nch[bl, 0]))]
                    chunks += [(1, int(qoff[bl, 1]) + j)
                               for j in range(int(nch[bl, 1]))]
                    pb = pblk.tile([128, 128], f32, tag="pb")
                    n_mm = len(chunks)
                    for i, (s, q) in enumerate(chunks):
                        seg, pos = q // SEGC, q % SEGC
                        if (s, seg) not in waited:
                            # keep LOOKAHEAD gathers in flight ahead of the
                            # consumer (criticals chain globally, so the
                            # consumer-side wait-critical throttles emission)
                            emit_ahead(seg_idx[(s, seg)] + LOOKAHEAD)
                            slot, uses = seg_slot[s][seg]
                            with tc.tile_critical():
                                nc.tensor.wait_ge(slot_sems[s][slot],
                                                  16 * uses)
                            waited.add((s, seg))
                        mt, st = tiles[(s, seg)]
                        nc.tensor.matmul(
                            pb[:], lhsT=mt[:, pos, :],
                            rhs=st[:, pos * 128:(pos + 1) * 128],
                            start=(i == 0), stop=(i == n_mm - 1))
                    nc.scalar.activation(a_T[:, bl * 128:(bl + 1) * 128],
                                         pb[:], Act.Copy)

                # ---- transform per col-tile (root+bias fused in psum)
                def transpose_tile(t):
                    pt = ptr_p.tile([128, 128], f32, tag="pt")
                    nc.tensor.transpose(pt[:], h_own[:, t * 128:(t + 1) * 128],
                                        ident_t[:])
                    hT = hT_pool.tile([128, 128], bf16, tag="h")
                    nc.scalar.activation(hT[:], pt[:], Act.Copy)
                    return hT

                hT_next = transpose_tile(0)
                for t in range(tpc):
                    hT = hT_next
                    if t + 1 < tpc:
                        hT_next = transpose_tile(t + 1)
                    po = pout.tile([128, 128], f32, tag="po")
                    for r in range(R):
                        bl = r * tpc + t
                        nc.tensor.matmul(
                            po[:], lhsT=a_T[:, bl * 128:(bl + 1) * 128],
                            rhs=w_t[:, (k * R + r) * D:(k * R + r + 1) * D],
                            start=(r == 0), stop=False)
                    nc.tensor.matmul(po[:], lhsT=hT[:],
                                     rhs=root_t[:, k * D:(k + 1) * D],
                                     start=False, stop=False)
                    nc.tensor.matmul(po[:], lhsT=ones_t[:],
                                     rhs=bias_t[:, k * D:(k + 1) * D],
                                     start=False, stop=True)
                    dst_sl = h_own[:, t * 128:(t + 1) * 128]
                    if k < k_layers - 1:
                        nc.scalar.activation(dst_sl, po[:], Act.Prelu,
                                             alpha=float(prelu_a))
                    else:
                        nc.scalar.activation(dst_sl, po[:], Act.Copy)

                # ---- export: cast + AllGather (not after last layer)
                if k < k_layers - 1:
                    nc.vector.tensor_copy(hbf[:], h_own[:])
                    nc.sync.dma_start(
                        ag_in.ap().rearrange("(t p) f -> p t f", p=128),
                        hbf[:].rearrange("p (t f) -> p t f", f=D))
                    nc.gpsimd.collective_compute(
                        "AllGather", Alu.bypass, replica_groups=rg,
                        ins=[ag_in.ap()], outs=[tables[k + 1].ap()])

        nc.sync.dma_start(out_own.ap().rearrange("(t p) f -> p t f", p=128),
                          h_own[:].rearrange("p (t f) -> p t f", f=D))

    nc.compile()
    return nc


def _host_tensors(cfg, sched, per_core, perms, inv_cnt, x, basis, att, root,
                  bias, k_layers=K):
    """Build in_maps for all cores."""
    ns, nsp, tpc = cfg.ns, cfg.nsp, cfg.tpc
    nstreams, nseg = sched["nstreams"], sched["nseg"]
    W = np.einsum("krb,kbio->krio", att.astype(np.float32),
                  basis.astype(np.float32))[:k_layers]  # [k,R,D,D]
    root = root[:k_layers]
    bias = bias[:k_layers]
    w_sw = np.ascontiguousarray(
        W.transpose(2, 0, 1, 3).reshape(D, k_layers * R * D)).astype(BF16)
    root_sw = np.ascontiguousarray(
        root.transpose(1, 0, 2).reshape(D, k_layers * D)).astype(BF16)
    bias_in = bias.reshape(1, k_layers * D).astype(BF16)
    ident = np.eye(128, dtype=np.float32)
    iota = np.tile(np.arange(128, dtype=np.float32), (128, 1)).astype(BF16)

    # global bf16 table [trows, D]
    table = np.zeros((cfg.trows, D), dtype=BF16)
    for c in range(cfg.ncores):
        sl = x[c * ns:(c + 1) * ns].astype(BF16)
        rowpos = c * nsp + perms[c]
        table[rowpos] = sl

    in_maps = []
    for c in range(cfg.ncores):
        x_own = np.zeros((128, nsp), dtype=np.float32)
        inv_perm = np.full(nsp, -1, dtype=np.int64)
        for v in range(ns):
            inv_perm[perms[c][v]] = v
        for t in range(tpc):
            vv = inv_perm[t * 128:(t + 1) * 128]
            ok = vv >= 0
            x_own[ok, t * 128:(t + 1) * 128] = x[c * ns + vv[ok]]
        pc = per_core[c]
        im = dict(x_table=table, x_own=x_own, w_sw=w_sw, root_sw=root_sw,
                  bias_in=bias_in, ident_in=ident, iota_in=iota)
        for s in range(nstreams):
            if nseg[s]:
                im[f"idx{s}"] = pc[f"idx{s}"]
                im[f"cv{s}"] = pc[f"cv{s}"]
                im[f"ivc{s}"] = pc[f"ivc{s}"]
        in_maps.append(im)
    return in_maps


def _run(cfg, x, edge_index, edge_attr, basis, att, root, bias, prelu_a,
         k_layers=K, trace=False, n_iter=1):
    from concourse.bass_utils import run_bass_kernel_spmd

    sched, per_core, perms, inv_cnt = _preprocess(cfg, edge_index, edge_attr)
    nc = _build_program(cfg, sched, k_layers,
                        float(np.asarray(prelu_a).ravel()[0]), n_iter=n_iter)
    in_maps = _host_tensors(cfg, sched, per_core, perms, inv_cnt,
                            np.asarray(x, dtype=np.float32),
                            np.asarray(basis), np.asarray(att),
                            np.asarray(root), np.asarray(bias), k_layers)
    res = run_bass_kernel_spmd(nc, in_maps, core_ids=list(range(cfg.ncores)),
                               trace=trace)
    out = np.empty((cfg.n, D), dtype=np.float32)
    for c in range(cfg.ncores):
        rows = res.results[c]["out_own"]  # [nsp, D] permuted
        out[c * cfg.ns:(c + 1) * cfg.ns] = rows[perms[c]]
    return out, res


def kernel(x, edge_index, edge_attr, basis, att, root, bias, prelu_a):
    cfg = Cfg()
    out, _ = _run(cfg, x, edge_index, edge_attr, basis, att, root, bias,
                  prelu_a)
    return out
